# revision 17
# baseline (speedup 1.0000x reference)
"""Multi-head attention (B=2, N=2048, D=1024, H=16) on 8 TRN2 NeuronCores.

Sharding: core c in 0..7 handles batch b=c//4 and head group hg=c%4 (4 heads
of 16).  Each core computes QKV for its heads, materialized attention, and a
partial projection (proj is row-split over heads); the host sums the 4
partials per batch and adds proj bias.  No device collectives.

Device layouts are feature-on-partition / tokens-on-free ("transposed"):
  xt   [1024, 2048]  x[b]^T                     bf16
  qk   [128 feats, 2048 toks] per feat-block    bf16  (QKV matmul + bias)
  vt   [128 toks, 4, 65] = [v_h | 1]            bf16  (ones col -> softmax sums)
  E^T  [128 Nk, 2*512] = exp(S^T * scale)       bf16  (ACT exp, psum->sbuf)
  AV   psum [65, 512]; row 64 = sums            fp32 accum
  out  [1024, 2048] partial (P_c @ O)^T         fp32  (proj in fp32r)

Pipeline: units = (head-pair, chunk).  Scores for unit i+1 are interleaved
kb-by-kb with AV matmuls of unit i so the ACT engine (exp is the global
floor, ~147us/core) never starves while PE runs AV.  Score matmuls for the
two heads of a pair go to different PE row groups (K=64) and run
concurrently.  V-stage and late-QK matmul groups act as PE filler during
the exp-bound prologue.
"""

import numpy as np

B, N, DIM, H, DH = 2, 2048, 1024, 16, 64
SCALE = DH ** -0.5
NCORE = 8
HPC = 4            # heads per core
F = HPC * DH       # 256 features per core-headgroup
CH = 512           # token chunk (matmul moving free dim)
NCH = N // CH      # 4
KT = DIM // 128    # 8 k-tiles over model dim
TB = N // 128      # 16 token blocks
_cache = {}


def _build():
    from contextlib import ExitStack

    import concourse.mybir as mybir
    from concourse import bacc
    from concourse.tile import TileContext

    f32 = mybir.dt.float32
    f32r = mybir.dt.float32r
    bf16 = mybir.dt.bfloat16
    nc = bacc.Bacc("TRN2", target_bir_lowering=False)

    xt_d = nc.declare_dram_parameter("xt", [DIM, N], bf16, isOutput=False)
    wqk_d = nc.declare_dram_parameter("wqk", [DIM, 2 * F], bf16, isOutput=False)
    wv_d = nc.declare_dram_parameter("wv", [DIM, F], bf16, isOutput=False)
    bqk_d = nc.declare_dram_parameter("bqk", [2 * F], f32, isOutput=False)
    bv_d = nc.declare_dram_parameter("bv", [F], f32, isOutput=False)
    pw_d = nc.declare_dram_parameter("pw", [F, DIM], f32r, isOutput=False)
    out_d = nc.declare_dram_parameter("out", [DIM, N], f32, isOutput=True)

    xt_r = xt_d.ap().rearrange("(t p) n -> t p n", p=128)
    wqk_r = wqk_d.ap().rearrange("(t p) m -> t p m", p=128)
    wv_r = wv_d.ap().rearrange("(t p) m -> t p m", p=128)
    pw_r = pw_d.ap().rearrange("(t p) m -> t p m", p=128)
    out_r = out_d.ap().rearrange("(t p) n -> t p n", p=128)

    with TileContext(nc) as tc, ExitStack() as st:
        consts = st.enter_context(tc.tile_pool(name="consts", bufs=1))
        qkp = st.enter_context(tc.tile_pool(name="qkp", bufs=1))
        vtp = st.enter_context(tc.tile_pool(name="vtp", bufs=1))
        otp = st.enter_context(tc.tile_pool(name="otp", bufs=1))
        ep = st.enter_context(tc.tile_pool(name="ep", bufs=2))
        recp = st.enter_context(tc.tile_pool(name="recp", bufs=3))
        outs = st.enter_context(tc.tile_pool(name="outs", bufs=3))
        stgp = st.enter_context(tc.tile_pool(name="stgp", bufs=4))
        xw = st.enter_context(tc.tile_pool(name="xw", bufs=1))
        ps_mm = st.enter_context(tc.tile_pool(name="ps_mm", bufs=2, space="PSUM"))
        ps_s = st.enter_context(tc.tile_pool(name="ps_s", bufs=2, space="PSUM"))
        ps_av = st.enter_context(tc.tile_pool(name="ps_av", bufs=2, space="PSUM"))

        bqk_sb = consts.tile([128, 2 * F // 128], f32)
        nc.sync.dma_start(out=bqk_sb, in_=bqk_d.ap().rearrange("(f p) -> p f", p=128))
        bv_sb = consts.tile([128, F], f32)
        nc.sync.dma_start(out=bv_sb, in_=bv_d.ap().partition_broadcast(128))
        pw_sb = [consts.tile([128, DIM], f32r, tag=f"pw{t}", name=f"pw{t}") for t in range(2)]
        for t in range(2):
            nc.sync.dma_start(out=pw_sb[t], in_=pw_r[t])
        ones64 = consts.tile([1, 64], f32)
        nc.vector.memset(ones64, 1.0)

        qk_sb = [qkp.tile([128, N], bf16, tag=f"qk{fb}", name=f"qk{fb}") for fb in range(4)]
        vt_sb = [vtp.tile([128, HPC, DH + 1], bf16, tag=f"vt{tb}", name=f"vt{tb}") for tb in range(TB)]
        ot_sb = [otp.tile([128, N], f32r, tag=f"ot{t}", name=f"ot{t}") for t in range(2)]

        xt_sb = [xw.tile([128, N], bf16, tag=f"x{t}", name=f"x{t}") for t in range(KT)]
        wqk_sb = [xw.tile([128, 2 * F], bf16, tag=f"wqk{t}", name=f"wqk{t}") for t in range(KT)]
        wv_sb = [xw.tile([128, F], bf16, tag=f"wv{t}", name=f"wv{t}") for t in range(KT)]
        for t in range(KT):
            nc.sync.dma_start(out=wqk_sb[t], in_=wqk_r[t])
            nc.sync.dma_start(out=wv_sb[t], in_=wv_r[t])
        for ch in range(NCH):
            for t in range(KT):
                nc.sync.dma_start(
                    out=xt_sb[t][:, ch * CH:(ch + 1) * CH],
                    in_=xt_r[t][:, ch * CH:(ch + 1) * CH],
                )

        def emit_qk_group(fb, ch):
            ps = ps_mm.tile([128, CH], f32, tag="mm", name=f"qkg{fb}_{ch}")
            for t in range(KT):
                nc.tensor.matmul(
                    ps,
                    wqk_sb[t][:, fb * 128:(fb + 1) * 128],
                    xt_sb[t][:, ch * CH:(ch + 1) * CH],
                    start=(t == 0), stop=(t == KT - 1),
                )
            nc.vector.tensor_scalar_add(
                out=qk_sb[fb][:, ch * CH:(ch + 1) * CH],
                in0=ps, scalar1=bqk_sb[:, fb:fb + 1],
            )

        def emit_v_group(tb):
            ps = ps_mm.tile([128, F], f32, tag="mm", name=f"vg{tb}")
            for t in range(KT):
                nc.tensor.matmul(
                    ps,
                    xt_sb[t][:, tb * 128:(tb + 1) * 128],
                    wv_sb[t],
                    start=(t == 0), stop=(t == KT - 1),
                )
            for hh in range(HPC):
                nc.vector.tensor_add(
                    out=vt_sb[tb][:, hh, :DH],
                    in0=ps[:, hh * DH:(hh + 1) * DH],
                    in1=bv_sb[:, hh * DH:(hh + 1) * DH],
                )
            nc.vector.memset(vt_sb[tb][:, :, DH:], 1.0)

        # minimal QK needed by the first score unit: all of k01 (fb2, every
        # chunk appears as contraction blocks) + q01 chunk 0 only
        for ch in range(NCH):
            emit_qk_group(2, ch)
        emit_qk_group(0, 0)

        # ---- pipelined attention units: unit = (head-pair hp, chunk cc) ----
        units = [(hp, cc) for hp in (0, 1) for cc in range(NCH)]
        et_store = {}

        def q_slice(h):
            return qk_sb[h // 2][(h % 2) * 64:(h % 2) * 64 + 64, :]

        def k_slice(h):
            return qk_sb[2 + h // 2][(h % 2) * 64:(h % 2) * 64 + 64, :]

        def emit_s(u, kb):
            hp, cc = u
            sp = ps_s.tile([128, 2 * CH], f32, tag="sp", name=f"sp{hp}_{cc}_{kb}")
            for j in range(2):
                h = 2 * hp + j
                nc.tensor.matmul(
                    sp[:, j * CH:(j + 1) * CH],
                    k_slice(h)[:, kb * 128:(kb + 1) * 128],
                    q_slice(h)[:, cc * CH:(cc + 1) * CH],
                    start=True, stop=True,
                )
            e = ep.tile([128, 2 * CH], bf16, tag=f"e{kb}", name=f"e{hp}_{cc}_{kb}")
            nc.scalar.activation(
                out=e, in_=sp,
                func=mybir.ActivationFunctionType.Exp, scale=SCALE,
            )
            et_store[u][kb] = e

        # prologue: scores for unit 0; V-stage + remaining q01 as PE filler
        pfill = [(0, 1), (0, 2), (0, 3)]
        et_store[units[0]] = [None] * TB
        for kb in range(TB):
            emit_s(units[0], kb)
            emit_v_group(kb)
            if kb % 4 == 3 and pfill:
                emit_qk_group(*pfill.pop(0))

        def emit_proj_group(fb, cc):
            ps = ps_mm.tile([128, CH], f32, tag="mm", name=f"pj{fb}_{cc}")
            for t in range(2):
                nc.tensor.matmul(
                    ps,
                    pw_sb[t][:, fb * 128:(fb + 1) * 128],
                    ot_sb[t][:, cc * CH:(cc + 1) * CH],
                    start=(t == 0), stop=(t == 1),
                )
            os = outs.tile([128, CH], f32, tag="os", name=f"os{fb}_{cc}")
            nc.vector.tensor_copy(out=os, in_=ps)
            nc.sync.dma_start(out=out_r[fb][:, cc * CH:(cc + 1) * CH], in_=os)

        # QK for heads 2/3: PE filler inside unit 0's AV block
        afill = [(fb, ch) for fb in (1, 3) for ch in range(NCH)]
        projq = []

        for i, u in enumerate(units):
            hp, cc = u
            nxt = units[i + 1] if i + 1 < len(units) else None
            if nxt is not None:
                et_store[nxt] = [None] * TB
            avs = [
                ps_av.tile([65, CH], f32, tag="av", name=f"av{hp}_{cc}_{j}")
                for j in range(2)
            ]
            for kb in range(TB):
                for j in range(2):
                    nc.tensor.matmul(
                        avs[j],
                        vt_sb[kb][:, 2 * hp + j, :],
                        et_store[u][kb][:, j * CH:(j + 1) * CH],
                        start=(kb == 0), stop=(kb == TB - 1),
                    )
                if nxt is not None:
                    emit_s(nxt, kb)
                if i == 0 and kb % 2 == 1 and afill:
                    emit_qk_group(*afill.pop(0))
                elif projq and kb % 2 == 1:
                    emit_proj_group(*projq.pop(0))
            et_store.pop(u)
            stgs = []
            for j in range(2):
                h = 2 * hp + j
                stg = stgp.tile([65, CH], f32, tag="stg", name=f"stg{h}_{cc}")
                nc.vector.tensor_copy(out=stg, in_=avs[j])
                stgs.append(stg)
            for j in range(2):
                h = 2 * hp + j
                stg = stgs[j]
                rec = recp.tile([1, CH], f32r, tag="rec", name=f"rec{h}_{cc}")
                with nc.allow_low_precision(reason="fp32r recip for softmax sums"):
                    nc.vector.reciprocal(out=rec, in_=stg[64:65, :])
                rb = ps_av.tile([64, CH], f32, tag="av", name=f"rb{h}_{cc}")
                nc.tensor.matmul(
                    rb, ones64.bitcast(f32r), rec, start=True, stop=True
                )
                nc.vector.tensor_mul(
                    out=ot_sb[h // 2][(h % 2) * 64:(h % 2) * 64 + 64,
                                      cc * CH:(cc + 1) * CH],
                    in0=stg[0:64, :], in1=rb,
                )
            if hp == 1:
                # defer this chunk's projection into the next unit's kb loop
                projq.extend((fb, cc) for fb in range(KT))

        for fb, cc in projq:
            emit_proj_group(fb, cc)

    nc.finalize()
    return nc


def _in_maps(x, qkv_w, qkv_b, proj_w):
    import ml_dtypes

    bf = ml_dtypes.bfloat16
    maps = []
    for c in range(NCORE):
        b, hg = c // 4, c % 4
        fs = slice(hg * F, (hg + 1) * F)
        wqk = np.concatenate([qkv_w[fs], qkv_w[DIM:][fs]], 0)        # [512,1024]
        bqk = np.concatenate([qkv_b[fs], qkv_b[DIM:][fs]], 0)
        maps.append({
            "xt": np.ascontiguousarray(x[b].T).astype(bf),
            "wqk": np.ascontiguousarray(wqk.T).astype(bf),
            "wv": np.ascontiguousarray(qkv_w[2 * DIM:][fs].T).astype(bf),
            "bqk": np.ascontiguousarray(bqk),
            "bv": np.ascontiguousarray(qkv_b[2 * DIM:][fs]),
            "pw": np.ascontiguousarray(proj_w[:, fs].T),
        })
    return maps


def _run(inputs, trace=False, trace_kwargs=None):
    from concourse.bass_utils import run_bass_kernel_spmd

    if "nc" not in _cache:
        _cache["nc"] = _build()
    nc = _cache["nc"]
    maps = _in_maps(inputs["x"], inputs["qkv_w"], inputs["qkv_b"], inputs["proj_w"])
    res = run_bass_kernel_spmd(
        nc, maps, list(range(NCORE)), trace=trace, **(trace_kwargs or {})
    )
    outs = [r["out"] for r in res.results]              # [1024, 2048] partials
    full = np.empty((B, N, DIM), dtype=np.float32)
    for b in range(B):
        acc = outs[4 * b].copy()
        for c in range(4 * b + 1, 4 * b + 4):
            acc += outs[c]
        full[b] = acc.T + inputs["proj_b"]
    return full, res


def kernel(**inputs) -> np.ndarray:
    out, _ = _run(inputs, trace=False)
    return out


# revision 18
# speedup vs baseline: 1.1112x; 1.1112x over previous
"""Multi-head attention (B=2, N=2048, D=1024, H=16) on 8 TRN2 NeuronCores.

Sharding: core c in 0..7 handles batch b=c//4 and head group hg=c%4 (4 heads
of 16).  Each core computes QKV for its heads, materialized attention, and a
partial projection (proj is row-split over heads); the host sums the 4
partials per batch and adds proj bias.  No device collectives.

Device layouts are feature-on-partition / tokens-on-free ("transposed"):
  xt   [1024, 2048]  x[b]^T                     bf16
  qk   [128 feats, 2048 toks] per feat-block    bf16  (QKV matmul + bias)
  vt   [128 toks, 4, 65] = [v_h | 1]            bf16  (ones col -> softmax sums)
  E^T  [128 Nk, 2*512] = exp(S^T * scale)       bf16  (ACT exp, psum->sbuf)
  AV   psum [65, 512]; row 64 = sums            fp32 accum
  out  [1024, 2048] partial (P_c @ O)^T         fp32  (proj in fp32r)

Pipeline: units = (head-pair, chunk).  Scores for unit i+1 are interleaved
kb-by-kb with AV matmuls of unit i so the ACT engine (exp is the global
floor, ~147us/core) never starves while PE runs AV.  Score matmuls for the
two heads of a pair go to different PE row groups (K=64) and run
concurrently.  V-stage and late-QK matmul groups act as PE filler during
the exp-bound prologue.
"""

import numpy as np

B, N, DIM, H, DH = 2, 2048, 1024, 16, 64
SCALE = DH ** -0.5
NCORE = 8
HPC = 4            # heads per core
F = HPC * DH       # 256 features per core-headgroup
CH = 512           # token chunk (matmul moving free dim)
NCH = N // CH      # 4
KT = DIM // 128    # 8 k-tiles over model dim
TB = N // 128      # 16 token blocks
_cache = {}


def _build():
    from contextlib import ExitStack

    import concourse.mybir as mybir
    from concourse import bacc
    from concourse.tile import TileContext

    f32 = mybir.dt.float32
    f32r = mybir.dt.float32r
    bf16 = mybir.dt.bfloat16
    nc = bacc.Bacc("TRN2", target_bir_lowering=False)

    xt_d = nc.declare_dram_parameter("xt", [DIM, N], bf16, isOutput=False)
    wqk_d = nc.declare_dram_parameter("wqk", [DIM, 2 * F], bf16, isOutput=False)
    wv_d = nc.declare_dram_parameter("wv", [DIM, F], bf16, isOutput=False)
    bqk_d = nc.declare_dram_parameter("bqk", [2 * F], f32, isOutput=False)
    bv_d = nc.declare_dram_parameter("bv", [F], f32, isOutput=False)
    pw_d = nc.declare_dram_parameter("pw", [F, DIM], f32r, isOutput=False)
    out_d = nc.declare_dram_parameter("out", [DIM, N], f32, isOutput=True)
    rscr = nc.dram_tensor("rscr", [HPC, NCH, CH], f32)

    xt_r = xt_d.ap().rearrange("(t p) n -> t p n", p=128)
    wqk_r = wqk_d.ap().rearrange("(t p) m -> t p m", p=128)
    wv_r = wv_d.ap().rearrange("(t p) m -> t p m", p=128)
    pw_r = pw_d.ap().rearrange("(t p) m -> t p m", p=128)
    out_r = out_d.ap().rearrange("(t p) n -> t p n", p=128)

    with TileContext(nc) as tc, ExitStack() as st:
        consts = st.enter_context(tc.tile_pool(name="consts", bufs=1))
        qkp = st.enter_context(tc.tile_pool(name="qkp", bufs=1))
        vtp = st.enter_context(tc.tile_pool(name="vtp", bufs=1))
        otp = st.enter_context(tc.tile_pool(name="otp", bufs=1))
        ep = st.enter_context(tc.tile_pool(name="ep", bufs=2))
        recp = st.enter_context(tc.tile_pool(name="recp", bufs=3))
        outs = st.enter_context(tc.tile_pool(name="outs", bufs=3))
        stgp = st.enter_context(tc.tile_pool(name="stgp", bufs=4))
        xw = st.enter_context(tc.tile_pool(name="xw", bufs=1))
        ps_mm = st.enter_context(tc.tile_pool(name="ps_mm", bufs=2, space="PSUM"))
        ps_s = st.enter_context(tc.tile_pool(name="ps_s", bufs=2, space="PSUM"))
        ps_av = st.enter_context(tc.tile_pool(name="ps_av", bufs=2, space="PSUM"))

        bqk_sb = consts.tile([128, 2 * F // 128], f32)
        nc.sync.dma_start(out=bqk_sb, in_=bqk_d.ap().rearrange("(f p) -> p f", p=128))
        bv_sb = consts.tile([128, F], f32)
        nc.sync.dma_start(out=bv_sb, in_=bv_d.ap().partition_broadcast(128))
        pw_sb = [consts.tile([128, DIM], f32r, tag=f"pw{t}", name=f"pw{t}") for t in range(2)]
        for t in range(2):
            nc.sync.dma_start(out=pw_sb[t], in_=pw_r[t])

        qk_sb = [qkp.tile([128, N], bf16, tag=f"qk{fb}", name=f"qk{fb}") for fb in range(4)]
        vt_sb = [vtp.tile([128, HPC, DH + 1], bf16, tag=f"vt{tb}", name=f"vt{tb}") for tb in range(TB)]
        ot_sb = [otp.tile([128, N], f32r, tag=f"ot{t}", name=f"ot{t}") for t in range(2)]

        xt_sb = [xw.tile([128, N], bf16, tag=f"x{t}", name=f"x{t}") for t in range(KT)]
        wqk_sb = [xw.tile([128, 2 * F], bf16, tag=f"wqk{t}", name=f"wqk{t}") for t in range(KT)]
        wv_sb = [xw.tile([128, F], bf16, tag=f"wv{t}", name=f"wv{t}") for t in range(KT)]
        for t in range(KT):
            nc.sync.dma_start(out=wqk_sb[t], in_=wqk_r[t])
            nc.sync.dma_start(out=wv_sb[t], in_=wv_r[t])
        for ch in range(NCH):
            for t in range(KT):
                nc.sync.dma_start(
                    out=xt_sb[t][:, ch * CH:(ch + 1) * CH],
                    in_=xt_r[t][:, ch * CH:(ch + 1) * CH],
                )

        def emit_qk_group(fb, ch):
            ps = ps_mm.tile([128, CH], f32, tag="mm", name=f"qkg{fb}_{ch}")
            for t in range(KT):
                nc.tensor.matmul(
                    ps,
                    wqk_sb[t][:, fb * 128:(fb + 1) * 128],
                    xt_sb[t][:, ch * CH:(ch + 1) * CH],
                    start=(t == 0), stop=(t == KT - 1),
                )
            nc.vector.tensor_scalar_add(
                out=qk_sb[fb][:, ch * CH:(ch + 1) * CH],
                in0=ps, scalar1=bqk_sb[:, fb:fb + 1],
            )

        def emit_v_group(tb):
            ps = ps_mm.tile([128, F], f32, tag="mm", name=f"vg{tb}")
            for t in range(KT):
                nc.tensor.matmul(
                    ps,
                    xt_sb[t][:, tb * 128:(tb + 1) * 128],
                    wv_sb[t],
                    start=(t == 0), stop=(t == KT - 1),
                )
            for hh in range(HPC):
                nc.vector.tensor_add(
                    out=vt_sb[tb][:, hh, :DH],
                    in0=ps[:, hh * DH:(hh + 1) * DH],
                    in1=bv_sb[:, hh * DH:(hh + 1) * DH],
                )
            nc.vector.memset(vt_sb[tb][:, :, DH:], 1.0)

        # minimal QK needed by the first score unit: all of k01 (fb2, every
        # chunk appears as contraction blocks) + q01 chunk 0 only
        for ch in range(NCH):
            emit_qk_group(2, ch)
        emit_qk_group(0, 0)

        # ---- pipelined attention units: unit = (head-pair hp, chunk cc) ----
        units = [(hp, cc) for hp in (0, 1) for cc in range(NCH)]
        et_store = {}

        def q_slice(h):
            return qk_sb[h // 2][(h % 2) * 64:(h % 2) * 64 + 64, :]

        def k_slice(h):
            return qk_sb[2 + h // 2][(h % 2) * 64:(h % 2) * 64 + 64, :]

        def emit_s(u, kb):
            hp, cc = u
            sp = ps_s.tile([128, 2 * CH], f32, tag="sp", name=f"sp{hp}_{cc}_{kb}")
            for j in range(2):
                h = 2 * hp + j
                nc.tensor.matmul(
                    sp[:, j * CH:(j + 1) * CH],
                    k_slice(h)[:, kb * 128:(kb + 1) * 128],
                    q_slice(h)[:, cc * CH:(cc + 1) * CH],
                    start=True, stop=True,
                )
            e = ep.tile([128, 2 * CH], bf16, tag=f"e{kb}", name=f"e{hp}_{cc}_{kb}")
            nc.scalar.activation(
                out=e, in_=sp,
                func=mybir.ActivationFunctionType.Exp, scale=SCALE,
            )
            et_store[u][kb] = e

        # prologue: scores for unit 0; V-stage + remaining q01 as PE filler
        pfill = [(0, 1), (0, 2), (0, 3)]
        et_store[units[0]] = [None] * TB
        for kb in range(TB):
            emit_s(units[0], kb)
            emit_v_group(kb)
            if kb % 4 == 3 and pfill:
                emit_qk_group(*pfill.pop(0))

        def emit_proj_group(fb, cc):
            ps = ps_mm.tile([128, CH], f32, tag="mm", name=f"pj{fb}_{cc}")
            for t in range(2):
                nc.tensor.matmul(
                    ps,
                    pw_sb[t][:, fb * 128:(fb + 1) * 128],
                    ot_sb[t][:, cc * CH:(cc + 1) * CH],
                    start=(t == 0), stop=(t == 1),
                )
            os = outs.tile([128, CH], f32, tag="os", name=f"os{fb}_{cc}")
            nc.vector.tensor_copy(out=os, in_=ps)
            nc.sync.dma_start(out=out_r[fb][:, cc * CH:(cc + 1) * CH], in_=os)

        # QK for heads 2/3: PE filler inside unit 0's AV block
        afill = [(fb, ch) for fb in (1, 3) for ch in range(NCH)]
        projq = []

        for i, u in enumerate(units):
            hp, cc = u
            nxt = units[i + 1] if i + 1 < len(units) else None
            if nxt is not None:
                et_store[nxt] = [None] * TB
            avs = [
                ps_av.tile([65, CH], f32, tag="av", name=f"av{hp}_{cc}_{j}")
                for j in range(2)
            ]
            for kb in range(TB):
                for j in range(2):
                    nc.tensor.matmul(
                        avs[j],
                        vt_sb[kb][:, 2 * hp + j, :],
                        et_store[u][kb][:, j * CH:(j + 1) * CH],
                        start=(kb == 0), stop=(kb == TB - 1),
                    )
                if nxt is not None:
                    emit_s(nxt, kb)
                if i == 0 and kb % 2 == 1 and afill:
                    emit_qk_group(*afill.pop(0))
                elif projq and 6 <= kb <= 13:
                    emit_proj_group(*projq.pop(0))
            et_store.pop(u)
            stgs = []
            for j in range(2):
                h = 2 * hp + j
                stg = stgp.tile([65, CH], f32, tag="stg", name=f"stg{h}_{cc}")
                nc.vector.tensor_copy(out=stg, in_=avs[j])
                stgs.append(stg)
            for j in range(2):
                h = 2 * hp + j
                stg = stgs[j]
                rec = recp.tile([1, CH], f32, tag="rec", name=f"rec{h}_{cc}")
                nc.vector.reciprocal(out=rec, in_=stg[64:65, :])
                nc.sync.dma_start(out=rscr.ap()[h, cc], in_=rec)
                rec64 = recp.tile([64, CH], f32, tag="rec64", name=f"rb{h}_{cc}")
                nc.sync.dma_start(
                    out=rec64, in_=rscr.ap()[h, cc].partition_broadcast(64)
                )
                nc.vector.tensor_mul(
                    out=ot_sb[h // 2][(h % 2) * 64:(h % 2) * 64 + 64,
                                      cc * CH:(cc + 1) * CH],
                    in0=stg[0:64, :], in1=rec64,
                )
            if hp == 1:
                # defer this chunk's projection into the next unit's kb loop
                projq.extend((fb, cc) for fb in range(KT))

        for fb, cc in projq:
            emit_proj_group(fb, cc)

    nc.finalize()
    return nc


def _in_maps(x, qkv_w, qkv_b, proj_w):
    import ml_dtypes

    bf = ml_dtypes.bfloat16
    maps = []
    for c in range(NCORE):
        b, hg = c // 4, c % 4
        fs = slice(hg * F, (hg + 1) * F)
        wqk = np.concatenate([qkv_w[fs], qkv_w[DIM:][fs]], 0)        # [512,1024]
        bqk = np.concatenate([qkv_b[fs], qkv_b[DIM:][fs]], 0)
        maps.append({
            "xt": np.ascontiguousarray(x[b].T).astype(bf),
            "wqk": np.ascontiguousarray(wqk.T).astype(bf),
            "wv": np.ascontiguousarray(qkv_w[2 * DIM:][fs].T).astype(bf),
            "bqk": np.ascontiguousarray(bqk),
            "bv": np.ascontiguousarray(qkv_b[2 * DIM:][fs]),
            "pw": np.ascontiguousarray(proj_w[:, fs].T),
        })
    return maps


def _run(inputs, trace=False, trace_kwargs=None):
    from concourse.bass_utils import run_bass_kernel_spmd

    if "nc" not in _cache:
        _cache["nc"] = _build()
    nc = _cache["nc"]
    maps = _in_maps(inputs["x"], inputs["qkv_w"], inputs["qkv_b"], inputs["proj_w"])
    res = run_bass_kernel_spmd(
        nc, maps, list(range(NCORE)), trace=trace, **(trace_kwargs or {})
    )
    outs = [r["out"] for r in res.results]              # [1024, 2048] partials
    full = np.empty((B, N, DIM), dtype=np.float32)
    for b in range(B):
        acc = outs[4 * b].copy()
        for c in range(4 * b + 1, 4 * b + 4):
            acc += outs[c]
        full[b] = acc.T + inputs["proj_b"]
    return full, res


def kernel(**inputs) -> np.ndarray:
    out, _ = _run(inputs, trace=False)
    return out
